# revision 3
# baseline (speedup 1.0000x reference)
"""Multi-head attention on 8 trn2 NeuronCores, head-parallel (2 heads/core).

Math per head h (reference semantics):
  Q = query @ Wq[h] + bq[h];  K = key @ Wk[h] + bk[h];  V = query @ Wv[h] + bv[h]
  P = exp(Q K^T / sqrt(D));  alpha = P / rowsum(P)
  ctx = alpha @ V;  y_h = (ctx @ Wp[h] + bp[h]) @ Wo[h]
  out = sum_h y_h + bo

Device-side formulation:
  Rows of alpha sum to 1, so all linear tails fold into the V projection:
    out = sum_h alpha_h @ (X Wv_h Wp_h Wo_h) + const_bias_row
  Per core: project QT/KT = W^T @ xT per head and V'' = X @ (Wv Wp Wo)
  (host-premultiplied), attention with unnormalized softmax (rowsum via
  ones-matmul), y = sum_{h in core} ctx_h / rowsum_h, ReduceScatter per
  1024-query block; host concatenates shards and adds the bias row.

Scheduling: the attention inner loop is ACT(exp)-bound while projections
are pure-PE, so projection work for batch b+1 is interleaved into the
attention kt-loops of batch b as "units" (one token-block of one
projection). This keeps the PE saturated (no HAM re-throttle micro-idles)
and hides the entire projection phase of batches 1..3.
"""

import sys

if "/opt/trn_rl_repo" not in sys.path:
    sys.path.insert(0, "/opt/trn_rl_repo")

from collections import deque

import ml_dtypes
import numpy as np

import concourse.mybir as mybir
import concourse.tile as tile
from concourse import bacc
from concourse.bass_utils import run_bass_kernel_spmd

B, S = 4, 2048
IN, D, H = 1024, 128, 16
NCORES = 8
HPC = H // NCORES  # heads per core
NCH = IN // 128  # input chunks
TB = 512  # projection token block
NTB = S // TB
QB = 512  # attention query block
NQB = S // QB
KT = 128  # attention key tile
NKT = S // KT
ESH = D // NCORES  # output shard rows per core

f32 = mybir.dt.float32
bf16 = mybir.dt.bfloat16
AF = mybir.ActivationFunctionType

_cache = {}


def build():
    nc = bacc.Bacc(None, target_bir_lowering=False, num_devices=NCORES)

    qT = nc.dram_tensor("qT", [B, IN, S], bf16, kind="ExternalInput")
    kT = nc.dram_tensor("kT", [B, IN, S], bf16, kind="ExternalInput")
    # prepacked [partition, head, chunk, d] so DMA loads are contiguous
    wq = nc.dram_tensor("wq", [128, HPC, NCH, D], bf16, kind="ExternalInput")
    wk = nc.dram_tensor("wk", [128, HPC, NCH, D], bf16, kind="ExternalInput")
    wv = nc.dram_tensor("wv", [128, HPC, NCH, D], bf16, kind="ExternalInput")
    bqT = nc.dram_tensor("bqT", [D, HPC], f32, kind="ExternalInput")
    bkT = nc.dram_tensor("bkT", [D, HPC], f32, kind="ExternalInput")
    onemb = nc.dram_tensor("onemb", [D, D], bf16, kind="ExternalInput")

    NQP = NQB // 2
    out_y = nc.dram_tensor("out_y", [B, NQP, ESH, 2 * QB], f32, kind="ExternalOutput")
    y_bounce = [
        [nc.dram_tensor(f"y_bounce{b}_{q}", [D, 2 * QB], f32) for q in range(NQP)]
        for b in range(B)
    ]
    y_shard = [
        [nc.dram_tensor(f"y_shard{b}_{q}", [ESH, 2 * QB], f32) for q in range(NQP)]
        for b in range(B)
    ]

    scale = 1.0 / float(np.sqrt(D))

    with tile.TileContext(nc) as tc:
        with (
            tc.tile_pool(name="const", bufs=1) as cpool,
            tc.tile_pool(name="xch", bufs=3) as xch,
            tc.tile_pool(name="qkv", bufs=2) as qkv,
            tc.tile_pool(name="work", bufs=2) as work,
            tc.tile_pool(name="pexpp", bufs=8) as pexpp,
            tc.tile_pool(name="psS", bufs=2, space="PSUM") as psS,
            tc.tile_pool(name="psC", bufs=1, space="PSUM") as psC,
            tc.tile_pool(name="psP", bufs=2, space="PSUM") as psP,
        ):
            # ---- resident constants, split per (h, c) for early start ----
            wq_sb = cpool.tile([128, HPC, NCH, D], bf16, tag="wq_sb")
            wk_sb = cpool.tile([128, HPC, NCH, D], bf16, tag="wk_sb")
            wv_sb = cpool.tile([128, HPC, NCH, D], bf16, tag="wv_sb")
            for h in range(HPC):
                for c in range(NCH):
                    for sb_t, dram_t in ((wq_sb, wq), (wk_sb, wk), (wv_sb, wv)):
                        nc.sync.dma_start(sb_t[:, h, c], dram_t[:, h, c])
            bq_sb = cpool.tile([128, HPC], f32, tag="bq_sb")
            bk_sb = cpool.tile([128, HPC], f32, tag="bk_sb")
            nc.sync.dma_start(bq_sb[:], bqT[:])
            nc.sync.dma_start(bk_sb[:], bkT[:])
            onemb_sb = cpool.tile([D, D], bf16, tag="onemb_sb")
            nc.sync.dma_start(onemb_sb[:], onemb[:])

            QTd, KTd, Vnd = {}, {}, {}

            # ---------- projection units ----------
            # Each unit: (batch, cls, dma_thunk, mm_thunk).  cls 'A' must land
            # before attn(batch) starts; cls 'B' (head-1 Q/K) before the first
            # h==1 attention section of the batch.
            def make_units(b):
                QT = QTd[b] = [qkv.tile([128, S], bf16, tag=f"QT{h}", name=f"QT{b}_{h}") for h in range(HPC)]
                KTs = KTd[b] = [qkv.tile([128, S], bf16, tag=f"KT{h}", name=f"KT{b}_{h}") for h in range(HPC)]
                Vn = Vnd[b] = [qkv.tile([128, S], bf16, tag=f"VN{h}", name=f"VN{b}_{h}") for h in range(HPC)]
                units = []

                def chunk_dma(src, tb):
                    chs = xch.tile([128, NCH, TB], bf16, tag="xch", bufs=3)
                    sl = slice(tb * TB, (tb + 1) * TB)
                    for c in range(NCH):
                        for p in range(2):
                            ps_ = slice(p * 64, (p + 1) * 64)
                            nc.sync.dma_start(
                                chs[ps_, c], src[b, c * 128 + p * 64 : c * 128 + (p + 1) * 64, sl]
                            )
                    return chs

                def k_unit(tb, h, box):
                    sl = slice(tb * TB, (tb + 1) * TB)
                    chs = box[0]
                    pk = psP.tile([128, TB], f32, tag="pP", name="pk", bufs=2)
                    for c in range(NCH):
                        nc.tensor.matmul(
                            pk[:], wk_sb[:, h, c, :], chs[:, c, :],
                            start=(c == 0), stop=(c == NCH - 1),
                        )
                    with nc.allow_low_precision(reason="f32 psum -> bf16"):
                        nc.vector.tensor_scalar_add(
                            KTs[h][:, sl], pk[:], bk_sb[:, h : h + 1]
                        )

                def q_unit(tb, h, box):
                    sl = slice(tb * TB, (tb + 1) * TB)
                    chs = box[0]
                    pq = psP.tile([128, TB], f32, tag="pP", name="pq", bufs=2)
                    for c in range(NCH):
                        nc.tensor.matmul(
                            pq[:], wq_sb[:, h, c, :], chs[:, c, :],
                            start=(c == 0), stop=(c == NCH - 1),
                        )
                    with nc.allow_low_precision(reason="f32 psum -> bf16"):
                        nc.vector.tensor_scalar_add(
                            QT[h][:, sl], pq[:], bq_sb[:, h : h + 1]
                        )

                def v_unit(tb, box):
                    chs = box[0]
                    for t in range(TB // 128):
                        pvt = psP.tile([128, 2 * D], f32, tag="pP", name="pvt", bufs=2)
                        for c in range(NCH):
                            nc.tensor.matmul(
                                pvt[:],
                                chs[:, c, t * 128 : (t + 1) * 128],
                                wv_sb[:, :, c, :],
                                start=(c == 0), stop=(c == NCH - 1),
                            )
                        col = tb * TB + t * 128
                        for h in range(HPC):
                            with nc.allow_low_precision(reason="bf16 PV operand"):
                                nc.vector.tensor_copy(
                                    Vn[h][:, col : col + 128],
                                    pvt[:, h * D : (h + 1) * D],
                                )

                def mk(cls, src, fns):
                    # fns: list of (fn, tb, ...) sharing one chunk tile
                    box = [None]
                    tb = fns[0][1]

                    def dma_thunk(box=box, src=src, tb=tb):
                        box[0] = chunk_dma(src, tb)

                    def mm_thunk(box=box, fns=fns):
                        for fn, tb, *rest in fns:
                            fn(tb, *rest, box)

                    units.append((b, cls, dma_thunk, mm_thunk))

                for tb in range(NTB):
                    mk("A", kT, [(k_unit, tb, 0)])
                for tb in range(NTB):
                    mk("A", qT, [(q_unit, tb, 0), (v_unit, tb)])
                for tb in range(NTB):
                    mk("B", kT, [(k_unit, tb, 1)])
                for tb in range(NTB):
                    mk("B", qT, [(q_unit, tb, 1)])
                return units

            # ---------- unit scheduler (dma issued 2 units ahead) ----------
            queue = deque()
            dma_lead = deque()  # units whose dma has been issued, mms pending
            LEAD = 2

            def _top_up():
                while queue and len(dma_lead) < LEAD:
                    u = queue.popleft()
                    u[2]()  # dma
                    dma_lead.append(u)

            def pull_one():
                _top_up()
                if dma_lead:
                    u = dma_lead.popleft()
                    u[3]()  # mms
                    _top_up()

            def flush(pred):
                _top_up()
                while dma_lead and pred(dma_lead[0]):
                    u = dma_lead.popleft()
                    u[3]()
                    _top_up()

            def push_units(b):
                for u in make_units(b):
                    queue.append(u)
                _top_up()

            # ---------- attention ----------
            def attn_batch(b):
                QT, KTs, Vn = QTd.pop(b), KTd.pop(b), Vnd.pop(b)
                for qbp in range(NQP):
                    q0 = qbp * 2 * QB
                    sl0 = slice(q0, q0 + QB)
                    sl1 = slice(q0 + QB, q0 + 2 * QB)
                    ytile = work.tile([128, 2 * QB], f32, tag="ytile", name="ytile")
                    for h in range(HPC):
                        if h == 1:
                            flush(lambda u: u[0] < b or (u[0] == b and u[1] == "B"))
                        pctx = psC.tile([128, 2 * QB], f32, tag="pCtx", name="pctx", bufs=1)
                        acc_d = work.tile([128, 2 * QB], bf16, tag="acc_d", name="acc_d")
                        st = [True, None]
                        for kt in range(NKT):
                            ps2 = psS.tile([128, 2 * QB], f32, tag="pS", name="ps2", bufs=2)
                            ksl = slice(kt * 128, (kt + 1) * 128)
                            nc.tensor.matmul(
                                ps2[:, :QB], KTs[h][:, ksl], QT[h][:, sl0],
                                start=True, stop=True,
                            )
                            nc.tensor.matmul(
                                ps2[:, QB:], KTs[h][:, ksl], QT[h][:, sl1],
                                start=True, stop=True,
                            )
                            pexp = pexpp.tile([128, 2 * QB], bf16, tag="pexp", bufs=8)
                            nc.scalar.activation(pexp[:], ps2[:], AF.Exp, scale=scale)
                            if kt % 3 == 1:
                                pull_one()  # proj filler where PE waits on exp
                            nc.tensor.matmul(
                                pctx[:, :QB], Vn[h][:, ksl], pexp[:, :QB],
                                start=(kt == 0), stop=(kt == NKT - 1),
                            )
                            nc.tensor.matmul(
                                pctx[:, QB:], Vn[h][:, ksl], pexp[:, QB:],
                                start=(kt == 0), stop=(kt == NKT - 1),
                            )
                            with nc.allow_low_precision(reason="bf16 rowsum acc"):
                                if st[0] and st[1] is None:
                                    st[1] = pexp
                                elif st[0]:
                                    nc.vector.tensor_add(acc_d[:], st[1][:], pexp[:])
                                    st[0] = False
                                else:
                                    nc.vector.tensor_add(acc_d[:], acc_d[:], pexp[:])
                        # rowsum collapse + normalize; boundary proj filler
                        # covers the PE while the DVE chain frees pctx.
                        rsbr = work.tile([128, 2 * QB], f32, tag="rsbr", name="rsbr", bufs=2)
                        for half in range(2):
                            hs = slice(half * QB, (half + 1) * QB)
                            pbc = psP.tile([128, QB], f32, tag="pP", name="pbc", bufs=2)
                            nc.tensor.matmul(
                                pbc[:], onemb_sb[:], acc_d[:, hs], start=True, stop=True
                            )
                            nc.vector.reciprocal_approx_fast(out=rsbr[:, hs], in_=pbc[:])
                        pull_one()
                        if h == 0:
                            for half in range(2):
                                hs = slice(half * QB, (half + 1) * QB)
                                nc.vector.tensor_mul(ytile[:, hs], pctx[:, hs], rsbr[:, hs])
                        else:
                            ctxn = work.tile([128, 2 * QB], f32, tag="ctxn", name="ctxn")
                            for half in range(2):
                                hs = slice(half * QB, (half + 1) * QB)
                                nc.vector.tensor_mul(ctxn[:, hs], pctx[:, hs], rsbr[:, hs])
                                nc.vector.tensor_add(
                                    ytile[:, hs], ytile[:, hs], ctxn[:, hs]
                                )
                                nc.gpsimd.dma_start(
                                    y_bounce[b][qbp][:, hs], ytile[:, hs]
                                )
                            nc.gpsimd.collective_compute(
                                "ReduceScatter",
                                mybir.AluOpType.add,
                                replica_groups=[list(range(NCORES))],
                                ins=[y_bounce[b][qbp][:].opt()],
                                outs=[y_shard[b][qbp][:].opt()],
                            )
                            nc.sync.dma_start(out_y[b, qbp], y_shard[b][qbp][:])

            # ---------- schedule ----------
            push_units(0)
            flush(lambda u: u[0] == 0 and u[1] == "A")
            for b in range(B):
                if b + 1 < B:
                    push_units(b + 1)
                attn_batch(b)
                flush(lambda u: u[0] <= b)
            flush(lambda u: True)

    nc.compile()
    return nc


def kernel(**inputs):
    query = np.asarray(inputs["query"], np.float32)
    key = np.asarray(inputs["key"], np.float32)
    Wq, bq = np.asarray(inputs["Wq"], np.float32), np.asarray(inputs["bq"], np.float32)
    Wk, bk = np.asarray(inputs["Wk"], np.float32), np.asarray(inputs["bk"], np.float32)
    Wv, bv = np.asarray(inputs["Wv"], np.float32), np.asarray(inputs["bv"], np.float32)
    Wp, bp = np.asarray(inputs["Wp"], np.float32), np.asarray(inputs["bp"], np.float32)
    Wo, bo = np.asarray(inputs["Wo"], np.float32), np.asarray(inputs["bo"], np.float32)

    qT_b16 = np.ascontiguousarray(query.transpose(0, 2, 1)).astype(ml_dtypes.bfloat16)
    kT_b16 = np.ascontiguousarray(key.transpose(0, 2, 1)).astype(ml_dtypes.bfloat16)

    if "nc" not in _cache:
        _cache["nc"] = build()
    nc = _cache["nc"]

    def prepack(w):  # [HPC, IN, D] -> [128, HPC, NCH, D] contiguous bf16
        return np.ascontiguousarray(
            w.reshape(HPC, NCH, 128, D).transpose(2, 0, 1, 3)
        ).astype(ml_dtypes.bfloat16)

    Wo_h = Wo.reshape(H, D, D)  # rows of Wo per head
    bias_total = (
        np.einsum("hd,hde,hef->f", bv.astype(np.float64), Wp.astype(np.float64), Wo_h.astype(np.float64))
        + np.einsum("hd,hdf->f", bp.astype(np.float64), Wo_h.astype(np.float64))
        + bo.astype(np.float64)
    ).astype(np.float32)

    in_maps = []
    for i in range(NCORES):
        hs = slice(i * HPC, (i + 1) * HPC)
        wvpp = np.einsum(
            "hid,hde,hef->hif",
            Wv[hs].astype(np.float64),
            Wp[hs].astype(np.float64),
            Wo_h[hs].astype(np.float64),
        ).astype(np.float32)
        in_maps.append(
            {
                "qT": qT_b16,
                "kT": kT_b16,
                "wq": prepack(Wq[hs]),
                "wk": prepack(Wk[hs]),
                "wv": prepack(wvpp),
                "bqT": np.ascontiguousarray(bq[hs].T),
                "bkT": np.ascontiguousarray(bk[hs].T),
                "onemb": np.ones((D, D), ml_dtypes.bfloat16),
            }
        )

    res = run_bass_kernel_spmd(nc, in_maps, core_ids=list(range(NCORES)))
    _cache["last_result"] = res
    # shards: per core [B, NQP, ESH, 2QB] -> full [B, S, D]
    parts = np.stack([res.results[i]["out_y"] for i in range(NCORES)], axis=2)
    # [B, NQP, NCORES, ESH, 2QB] -> [B, NQP, 2QB, D] -> [B, S, D]
    NQP = NQB // 2
    yfull = parts.reshape(B, NQP, D, 2 * QB).transpose(0, 1, 3, 2).reshape(B, S, D)
    return np.ascontiguousarray(yfull + bias_total[None, None, :])


# revision 5
# speedup vs baseline: 1.4610x; 1.4610x over previous
"""Multi-head attention on 8 trn2 NeuronCores, head-parallel (2 heads/core).

Math per head h (reference semantics):
  Q = query @ Wq[h] + bq[h];  K = key @ Wk[h] + bk[h];  V = query @ Wv[h] + bv[h]
  P = exp(Q K^T / sqrt(D));  alpha = P / rowsum(P)
  ctx = alpha @ V;  y_h = (ctx @ Wp[h] + bp[h]) @ Wo[h]
  out = sum_h y_h + bo

Device-side formulation:
  Rows of alpha sum to 1, so all linear tails fold into the V projection:
    out = sum_h alpha_h @ (X Wv_h Wp_h Wo_h) + const_bias_row
  Per core: project QT/KT = W^T @ xT per head and V'' = X @ (Wv Wp Wo)
  (host-premultiplied), attention with unnormalized softmax (rowsum via
  ones-matmul), y = sum_{h in core} ctx_h / rowsum_h, ReduceScatter per
  1024-query block; host concatenates shards and adds the bias row.

Scheduling: the attention inner loop is ACT(exp)-bound while projections
are pure-PE, so projection work for batch b+1 is interleaved into the
attention kt-loops of batch b as "units" (one token-block of one
projection). This keeps the PE saturated (no HAM re-throttle micro-idles)
and hides the entire projection phase of batches 1..3.
"""

import sys

if "/opt/trn_rl_repo" not in sys.path:
    sys.path.insert(0, "/opt/trn_rl_repo")

from collections import deque

import ml_dtypes
import numpy as np

import concourse.mybir as mybir
import concourse.tile as tile
from concourse import bacc
from concourse.bass_utils import run_bass_kernel_spmd

B, S = 4, 2048
IN, D, H = 1024, 128, 16
NCORES = 8
HPC = H // NCORES  # heads per core
NCH = IN // 128  # input chunks
TB = 512  # projection token block
NTB = S // TB
QB = 512  # attention query block
NQB = S // QB
KT = 128  # attention key tile
NKT = S // KT
ESH = D // NCORES  # output shard rows per core

f32 = mybir.dt.float32
bf16 = mybir.dt.bfloat16
AF = mybir.ActivationFunctionType

_cache = {}


def build():
    nc = bacc.Bacc(None, target_bir_lowering=False, num_devices=NCORES)

    qT = nc.dram_tensor("qT", [B, IN, S], bf16, kind="ExternalInput")
    kT = nc.dram_tensor("kT", [B, IN, S], bf16, kind="ExternalInput")
    # prepacked [partition, head, chunk, d] so DMA loads are contiguous
    wq = nc.dram_tensor("wq", [128, HPC, NCH, D], bf16, kind="ExternalInput")
    wk = nc.dram_tensor("wk", [128, HPC, NCH, D], bf16, kind="ExternalInput")
    wv = nc.dram_tensor("wv", [128, HPC, NCH, D], bf16, kind="ExternalInput")
    bqT = nc.dram_tensor("bqT", [D, HPC], f32, kind="ExternalInput")
    bkT = nc.dram_tensor("bkT", [D, HPC], f32, kind="ExternalInput")
    onemb = nc.dram_tensor("onemb", [D, D], bf16, kind="ExternalInput")

    NQP = NQB // 2
    out_y = nc.dram_tensor("out_y", [B, NQP, ESH, 2 * QB], f32, kind="ExternalOutput")
    y_bounce = [
        [nc.dram_tensor(f"y_bounce{b}_{q}", [D, 2 * QB], f32) for q in range(NQP)]
        for b in range(B)
    ]
    y_shard = [
        [nc.dram_tensor(f"y_shard{b}_{q}", [ESH, 2 * QB], f32) for q in range(NQP)]
        for b in range(B)
    ]

    scale = 1.0 / float(np.sqrt(D))

    with tile.TileContext(nc) as tc:
        with (
            tc.tile_pool(name="const", bufs=1) as cpool,
            tc.tile_pool(name="xch", bufs=3) as xch,
            tc.tile_pool(name="qkv", bufs=2) as qkv,
            tc.tile_pool(name="work", bufs=2) as work,
            tc.tile_pool(name="pexpp", bufs=8) as pexpp,
            tc.tile_pool(name="psS", bufs=2, space="PSUM") as psS,
            tc.tile_pool(name="psC", bufs=1, space="PSUM") as psC,
            tc.tile_pool(name="psP", bufs=2, space="PSUM") as psP,
        ):
            # ---- resident constants (one DMA op each: issue cost on the
            # sync sequencer is ~600ns per dma_start, so keep op count low;
            # a single op fans out across all 16 DMA queues) ----
            wq_sb = cpool.tile([128, HPC, NCH, D], bf16, tag="wq_sb")
            wk_sb = cpool.tile([128, HPC, NCH, D], bf16, tag="wk_sb")
            wv_sb = cpool.tile([128, HPC, NCH, D], bf16, tag="wv_sb")
            for sb_t, dram_t in ((wq_sb, wq), (wk_sb, wk), (wv_sb, wv)):
                nc.sync.dma_start(sb_t[:], dram_t[:])
            bq_sb = cpool.tile([128, HPC], f32, tag="bq_sb")
            bk_sb = cpool.tile([128, HPC], f32, tag="bk_sb")
            nc.sync.dma_start(bq_sb[:], bqT[:])
            nc.sync.dma_start(bk_sb[:], bkT[:])
            onemb_sb = cpool.tile([D, D], bf16, tag="onemb_sb")
            nc.sync.dma_start(onemb_sb[:], onemb[:])

            QTd, KTd, Vnd = {}, {}, {}

            # ---------- projection units ----------
            # Each unit: (batch, cls, dma_thunk, mm_thunk).  cls 'A' must land
            # before attn(batch) starts; cls 'B' (head-1 Q/K) before the first
            # h==1 attention section of the batch.
            def make_units(b):
                QT = QTd[b] = [qkv.tile([128, S], bf16, tag=f"QT{h}", name=f"QT{b}_{h}") for h in range(HPC)]
                KTs = KTd[b] = [qkv.tile([128, S], bf16, tag=f"KT{h}", name=f"KT{b}_{h}") for h in range(HPC)]
                Vn = Vnd[b] = [qkv.tile([128, S], bf16, tag=f"VN{h}", name=f"VN{b}_{h}") for h in range(HPC)]
                units = []

                def chunk_dma(src, tb):
                    chs = xch.tile([128, NCH, TB], bf16, tag="xch", bufs=3)
                    sl = slice(tb * TB, (tb + 1) * TB)
                    nc.sync.dma_start(
                        chs[:], src[b, :, sl].rearrange("(c p) n -> p c n", p=128)
                    )
                    return chs

                def k_unit(tb, h, box):
                    sl = slice(tb * TB, (tb + 1) * TB)
                    chs = box[0]
                    pk = psP.tile([128, TB], f32, tag="pP", name="pk", bufs=2)
                    for c in range(NCH):
                        nc.tensor.matmul(
                            pk[:], wk_sb[:, h, c, :], chs[:, c, :],
                            start=(c == 0), stop=(c == NCH - 1),
                        )
                    with nc.allow_low_precision(reason="f32 psum -> bf16"):
                        nc.vector.tensor_scalar_add(
                            KTs[h][:, sl], pk[:], bk_sb[:, h : h + 1]
                        )

                def q_unit(tb, h, box):
                    sl = slice(tb * TB, (tb + 1) * TB)
                    chs = box[0]
                    pq = psP.tile([128, TB], f32, tag="pP", name="pq", bufs=2)
                    for c in range(NCH):
                        nc.tensor.matmul(
                            pq[:], wq_sb[:, h, c, :], chs[:, c, :],
                            start=(c == 0), stop=(c == NCH - 1),
                        )
                    with nc.allow_low_precision(reason="f32 psum -> bf16"):
                        nc.vector.tensor_scalar_add(
                            QT[h][:, sl], pq[:], bq_sb[:, h : h + 1]
                        )

                def v_unit(tb, box):
                    chs = box[0]
                    for t in range(TB // 128):
                        pvt = psP.tile([128, 2 * D], f32, tag="pP", name="pvt", bufs=2)
                        for c in range(NCH):
                            nc.tensor.matmul(
                                pvt[:],
                                chs[:, c, t * 128 : (t + 1) * 128],
                                wv_sb[:, :, c, :],
                                start=(c == 0), stop=(c == NCH - 1),
                            )
                        col = tb * TB + t * 128
                        for h in range(HPC):
                            with nc.allow_low_precision(reason="bf16 PV operand"):
                                nc.vector.tensor_copy(
                                    Vn[h][:, col : col + 128],
                                    pvt[:, h * D : (h + 1) * D],
                                )

                def mk(cls, src, fns):
                    # fns: list of (fn, tb, ...) sharing one chunk tile
                    box = [None]
                    tb = fns[0][1]

                    def dma_thunk(box=box, src=src, tb=tb):
                        box[0] = chunk_dma(src, tb)

                    def mm_thunk(box=box, fns=fns):
                        for fn, tb, *rest in fns:
                            fn(tb, *rest, box)

                    units.append((b, cls, dma_thunk, mm_thunk))

                for tb in range(NTB):
                    mk("A", kT, [(k_unit, tb, 0)])
                for tb in range(NTB):
                    mk("A", qT, [(q_unit, tb, 0), (v_unit, tb)])
                for tb in range(NTB):
                    mk("B", kT, [(k_unit, tb, 1)])
                for tb in range(NTB):
                    mk("B", qT, [(q_unit, tb, 1)])
                return units

            # ---------- unit scheduler (dma issued 2 units ahead) ----------
            queue = deque()
            dma_lead = deque()  # units whose dma has been issued, mms pending
            LEAD = 2

            def _top_up():
                while queue and len(dma_lead) < LEAD:
                    u = queue.popleft()
                    u[2]()  # dma
                    dma_lead.append(u)

            def pull_one():
                _top_up()
                if dma_lead:
                    u = dma_lead.popleft()
                    u[3]()  # mms
                    _top_up()

            def flush(pred):
                _top_up()
                while dma_lead and pred(dma_lead[0]):
                    u = dma_lead.popleft()
                    u[3]()
                    _top_up()

            def push_units(b):
                for u in make_units(b):
                    queue.append(u)
                _top_up()

            # ---------- attention ----------
            def attn_batch(b):
                QT, KTs, Vn = QTd.pop(b), KTd.pop(b), Vnd.pop(b)
                for qbp in range(NQP):
                    q0 = qbp * 2 * QB
                    sl0 = slice(q0, q0 + QB)
                    sl1 = slice(q0 + QB, q0 + 2 * QB)
                    ytile = work.tile([128, 2 * QB], f32, tag="ytile", name="ytile")
                    for h in range(HPC):
                        if h == 1:
                            flush(lambda u: u[0] < b or (u[0] == b and u[1] == "B"))
                        pctx = psC.tile([128, 2 * QB], f32, tag="pCtx", name="pctx", bufs=1)
                        acc_d = work.tile([128, 2 * QB], bf16, tag="acc_d", name="acc_d")
                        st = [True, None]
                        for kt in range(NKT):
                            ps2 = psS.tile([128, 2 * QB], f32, tag="pS", name="ps2", bufs=2)
                            ksl = slice(kt * 128, (kt + 1) * 128)
                            nc.tensor.matmul(
                                ps2[:, :QB], KTs[h][:, ksl], QT[h][:, sl0],
                                start=True, stop=True,
                            )
                            nc.tensor.matmul(
                                ps2[:, QB:], KTs[h][:, ksl], QT[h][:, sl1],
                                start=True, stop=True,
                            )
                            pexp = pexpp.tile([128, 2 * QB], bf16, tag="pexp", bufs=8)
                            nc.scalar.activation(pexp[:], ps2[:], AF.Exp, scale=scale)
                            if kt % 3 == 1:
                                pull_one()  # proj filler where PE waits on exp
                            nc.tensor.matmul(
                                pctx[:, :QB], Vn[h][:, ksl], pexp[:, :QB],
                                start=(kt == 0), stop=(kt == NKT - 1),
                            )
                            nc.tensor.matmul(
                                pctx[:, QB:], Vn[h][:, ksl], pexp[:, QB:],
                                start=(kt == 0), stop=(kt == NKT - 1),
                            )
                            with nc.allow_low_precision(reason="bf16 rowsum acc"):
                                if st[0] and st[1] is None:
                                    st[1] = pexp
                                elif st[0]:
                                    nc.vector.tensor_add(acc_d[:], st[1][:], pexp[:])
                                    st[0] = False
                                else:
                                    nc.vector.tensor_add(acc_d[:], acc_d[:], pexp[:])
                        # rowsum collapse + normalize; boundary proj filler
                        # covers the PE while the DVE chain frees pctx.
                        rsbr = work.tile([128, 2 * QB], f32, tag="rsbr", name="rsbr", bufs=2)
                        for half in range(2):
                            hs = slice(half * QB, (half + 1) * QB)
                            pbc = psP.tile([128, QB], f32, tag="pP", name="pbc", bufs=2)
                            nc.tensor.matmul(
                                pbc[:], onemb_sb[:], acc_d[:, hs], start=True, stop=True
                            )
                            nc.vector.reciprocal_approx_fast(out=rsbr[:, hs], in_=pbc[:])
                        pull_one()
                        if h == 0:
                            for half in range(2):
                                hs = slice(half * QB, (half + 1) * QB)
                                nc.vector.tensor_mul(ytile[:, hs], pctx[:, hs], rsbr[:, hs])
                        else:
                            ctxn = work.tile([128, 2 * QB], f32, tag="ctxn", name="ctxn")
                            for half in range(2):
                                hs = slice(half * QB, (half + 1) * QB)
                                nc.vector.tensor_mul(ctxn[:, hs], pctx[:, hs], rsbr[:, hs])
                                nc.vector.tensor_add(
                                    ytile[:, hs], ytile[:, hs], ctxn[:, hs]
                                )
                                nc.gpsimd.dma_start(
                                    y_bounce[b][qbp][:, hs], ytile[:, hs]
                                )
                            nc.gpsimd.collective_compute(
                                "ReduceScatter",
                                mybir.AluOpType.add,
                                replica_groups=[list(range(NCORES))],
                                ins=[y_bounce[b][qbp][:].opt()],
                                outs=[y_shard[b][qbp][:].opt()],
                            )
                            nc.sync.dma_start(out_y[b, qbp], y_shard[b][qbp][:])

            # ---------- schedule ----------
            push_units(0)
            flush(lambda u: u[0] == 0 and u[1] == "A")
            for b in range(B):
                if b + 1 < B:
                    push_units(b + 1)
                attn_batch(b)
                flush(lambda u: u[0] <= b)
            flush(lambda u: True)

    nc.compile()
    return nc


def kernel(**inputs):
    query = np.asarray(inputs["query"], np.float32)
    key = np.asarray(inputs["key"], np.float32)
    Wq, bq = np.asarray(inputs["Wq"], np.float32), np.asarray(inputs["bq"], np.float32)
    Wk, bk = np.asarray(inputs["Wk"], np.float32), np.asarray(inputs["bk"], np.float32)
    Wv, bv = np.asarray(inputs["Wv"], np.float32), np.asarray(inputs["bv"], np.float32)
    Wp, bp = np.asarray(inputs["Wp"], np.float32), np.asarray(inputs["bp"], np.float32)
    Wo, bo = np.asarray(inputs["Wo"], np.float32), np.asarray(inputs["bo"], np.float32)

    qT_b16 = np.ascontiguousarray(query.transpose(0, 2, 1)).astype(ml_dtypes.bfloat16)
    kT_b16 = np.ascontiguousarray(key.transpose(0, 2, 1)).astype(ml_dtypes.bfloat16)

    if "nc" not in _cache:
        _cache["nc"] = build()
    nc = _cache["nc"]

    def prepack(w):  # [HPC, IN, D] -> [128, HPC, NCH, D] contiguous bf16
        return np.ascontiguousarray(
            w.reshape(HPC, NCH, 128, D).transpose(2, 0, 1, 3)
        ).astype(ml_dtypes.bfloat16)

    Wo_h = Wo.reshape(H, D, D)  # rows of Wo per head
    bias_total = (
        np.einsum("hd,hde,hef->f", bv.astype(np.float64), Wp.astype(np.float64), Wo_h.astype(np.float64))
        + np.einsum("hd,hdf->f", bp.astype(np.float64), Wo_h.astype(np.float64))
        + bo.astype(np.float64)
    ).astype(np.float32)

    in_maps = []
    for i in range(NCORES):
        hs = slice(i * HPC, (i + 1) * HPC)
        wvpp = np.einsum(
            "hid,hde,hef->hif",
            Wv[hs].astype(np.float64),
            Wp[hs].astype(np.float64),
            Wo_h[hs].astype(np.float64),
        ).astype(np.float32)
        in_maps.append(
            {
                "qT": qT_b16,
                "kT": kT_b16,
                "wq": prepack(Wq[hs]),
                "wk": prepack(Wk[hs]),
                "wv": prepack(wvpp),
                "bqT": np.ascontiguousarray(bq[hs].T),
                "bkT": np.ascontiguousarray(bk[hs].T),
                "onemb": np.ones((D, D), ml_dtypes.bfloat16),
            }
        )

    res = run_bass_kernel_spmd(nc, in_maps, core_ids=list(range(NCORES)))
    _cache["last_result"] = res
    # shards: per core [B, NQP, ESH, 2QB] -> full [B, S, D]
    parts = np.stack([res.results[i]["out_y"] for i in range(NCORES)], axis=2)
    # [B, NQP, NCORES, ESH, 2QB] -> [B, NQP, 2QB, D] -> [B, S, D]
    NQP = NQB // 2
    yfull = parts.reshape(B, NQP, D, 2 * QB).transpose(0, 1, 3, 2).reshape(B, S, D)
    return np.ascontiguousarray(yfull + bias_total[None, None, :])


# revision 7
# speedup vs baseline: 1.5815x; 1.0825x over previous
"""Multi-head attention on 8 trn2 NeuronCores, head-parallel (2 heads/core).

Math per head h (reference semantics):
  Q = query @ Wq[h] + bq[h];  K = key @ Wk[h] + bk[h];  V = query @ Wv[h] + bv[h]
  P = exp(Q K^T / sqrt(D));  alpha = P / rowsum(P)
  ctx = alpha @ V;  y_h = (ctx @ Wp[h] + bp[h]) @ Wo[h]
  out = sum_h y_h + bo

Device-side formulation:
  Rows of alpha sum to 1, so all linear tails fold into the V projection:
    out = sum_h alpha_h @ (X Wv_h Wp_h Wo_h) + const_bias_row
  Projections and the PV contraction run in fp8-e4m3 with DoubleRow perf
  mode (2 k-tiles per pass = 2x PE throughput); QK^T stays bf16 (its
  contraction is a single 128 k-tile).  Weights are host-scaled into the
  fp8 normal range (wq,wk x32 -> exp scale /1024; wv''=Wv Wp Wo x64 ->
  host output /64).  Unnormalized softmax; rowsum via ones-matmul
  collapse of a DVE-accumulated exp sum; y = sum_h ctx_h / rowsum_h;
  ReduceScatter per 1024-query block; host adds the bias row.

Scheduling: the attention loop is ACT(exp)-bound while projections are
pure-PE, so projection work for batch b+1 is interleaved into the
attention kt-loops of batch b as "units".  DMA issue cost (~600ns per
dma_start on the issuing sequencer) means few, large dma ops.  Queues:
input chunks on sync, y staging on vector, collectives + out dma on
gpsimd (so a waiting ReduceScatter never blocks input DMAs).
"""

import sys

if "/opt/trn_rl_repo" not in sys.path:
    sys.path.insert(0, "/opt/trn_rl_repo")

from collections import deque

import ml_dtypes
import numpy as np

import concourse.mybir as mybir
import concourse.tile as tile
from concourse import bacc
from concourse.bass_utils import run_bass_kernel_spmd

B, S = 4, 2048
IN, D, H = 1024, 128, 16
NCORES = 8
HPC = H // NCORES  # heads per core
NCH = IN // 128  # input chunks
TB = 512  # projection token block
NTB = S // TB
QB = 512  # attention query block
NQB = S // QB
NKT = S // 128  # attention key tiles
NKP = NKT // 2  # key-tile pairs (DoubleRow)
ESH = D // NCORES  # output shard rows per core
NQP = NQB // 2

f32 = mybir.dt.float32
bf16 = mybir.dt.bfloat16
f8 = mybir.dt.float8e4
AF = mybir.ActivationFunctionType
DR = mybir.MatmulPerfMode.DoubleRow

_cache = {}


def build():
    nc = bacc.Bacc(None, target_bir_lowering=False, num_devices=NCORES)

    qT = nc.dram_tensor("qT", [B, IN, S], f8, kind="ExternalInput")
    kT = nc.dram_tensor("kT", [B, IN, S], f8, kind="ExternalInput")
    # prepacked, host-scaled fp8 weights (see kernel())
    wq = nc.dram_tensor("wq", [128, HPC, NCH, D], f8, kind="ExternalInput")
    wk = nc.dram_tensor("wk", [128, HPC, NCH, D], f8, kind="ExternalInput")
    wv = nc.dram_tensor("wv", [128, NCH, HPC, D], f8, kind="ExternalInput")
    bqT = nc.dram_tensor("bqT", [D, HPC], f32, kind="ExternalInput")
    bkT = nc.dram_tensor("bkT", [D, HPC], f32, kind="ExternalInput")
    onemb = nc.dram_tensor("onemb", [D, D], bf16, kind="ExternalInput")

    out_y = nc.dram_tensor("out_y", [B, NQP, ESH, 2 * QB], f32, kind="ExternalOutput")
    y_bounce = [
        [nc.dram_tensor(f"y_bounce{b}_{q}", [D, 2 * QB], f32) for q in range(NQP)]
        for b in range(B)
    ]
    y_shard = [
        [nc.dram_tensor(f"y_shard{b}_{q}", [ESH, 2 * QB], f32) for q in range(NQP)]
        for b in range(B)
    ]

    # Q,K are x32-scaled -> scores x1024
    scale = 1.0 / float(np.sqrt(D)) / 1024.0

    with tile.TileContext(nc) as tc:
        with (
            tc.tile_pool(name="const", bufs=1) as cpool,
            tc.tile_pool(name="xch", bufs=3) as xch,
            tc.tile_pool(name="qkv", bufs=2) as qkv,
            tc.tile_pool(name="work", bufs=2) as work,
            tc.tile_pool(name="pexpp", bufs=6) as pexpp,
            tc.tile_pool(name="psS", bufs=2, space="PSUM") as psS,
            tc.tile_pool(name="psC", bufs=1, space="PSUM") as psC,
            tc.tile_pool(name="psP", bufs=2, space="PSUM") as psP,
        ):
            # ---- resident constants (one DMA op each) ----
            wq_sb = cpool.tile([128, HPC, NCH, D], f8, tag="wq_sb")
            wk_sb = cpool.tile([128, HPC, NCH, D], f8, tag="wk_sb")
            wv_sb = cpool.tile([128, NCH, HPC, D], f8, tag="wv_sb")
            for sb_t, dram_t in ((wq_sb, wq), (wk_sb, wk), (wv_sb, wv)):
                nc.sync.dma_start(sb_t[:], dram_t[:])
            bq_sb = cpool.tile([128, HPC], f32, tag="bq_sb")
            bk_sb = cpool.tile([128, HPC], f32, tag="bk_sb")
            nc.sync.dma_start(bq_sb[:], bqT[:])
            nc.sync.dma_start(bk_sb[:], bkT[:])
            onemb_sb = cpool.tile([D, D], bf16, tag="onemb_sb")
            nc.sync.dma_start(onemb_sb[:], onemb[:])

            QTd, KTd, Vnd = {}, {}, {}

            # ---------- projection units ----------
            # Each unit: (batch, cls, dma_thunk, mm_thunk).  cls 'A' must land
            # before attn(batch) starts; cls 'B' (head-1 Q/K) before the first
            # h==1 attention section of the batch.
            def make_units(b):
                QT = QTd[b] = [qkv.tile([128, S], bf16, tag=f"QT{h}", name=f"QT{b}_{h}") for h in range(HPC)]
                KTs = KTd[b] = [qkv.tile([128, S], bf16, tag=f"KT{h}", name=f"KT{b}_{h}") for h in range(HPC)]
                Vn = Vnd[b] = [
                    qkv.tile([128, NKP, 2, 128], f8, tag=f"VN{h}", name=f"VN{b}_{h}")
                    for h in range(HPC)
                ]
                units = []

                def chunk_dma(src, tb):
                    chs = xch.tile([128, NCH, TB], f8, tag="xch", bufs=3)
                    sl = slice(tb * TB, (tb + 1) * TB)
                    nc.sync.dma_start(
                        chs[:], src[b, :, sl].rearrange("(c p) n -> p c n", p=128)
                    )
                    return chs

                def qk_unit(tb, h, w_sb, bias_sb, dst, box):
                    sl = slice(tb * TB, (tb + 1) * TB)
                    chs = box[0]
                    pq = psP.tile([128, TB], f32, tag="pP", name="pqk", bufs=2)
                    for cp in range(NCH // 2):
                        nc.tensor.matmul(
                            pq[:],
                            w_sb[:, h, 2 * cp : 2 * cp + 2, :],
                            chs[:, 2 * cp : 2 * cp + 2, :],
                            start=(cp == 0), stop=(cp == NCH // 2 - 1),
                            perf_mode=DR,
                        )
                    with nc.allow_low_precision(reason="f32 psum -> bf16"):
                        nc.vector.tensor_scalar_add(
                            dst[h][:, sl], pq[:], bias_sb[:, h : h + 1]
                        )

                def k_unit(tb, h, box):
                    qk_unit(tb, h, wk_sb, bk_sb, KTs, box)

                def q_unit(tb, h, box):
                    qk_unit(tb, h, wq_sb, bq_sb, QT, box)

                def v_unit(tb, box):
                    chs = box[0]
                    for t in range(TB // 128):
                        pvt = psP.tile([128, 2 * D], f32, tag="pP", name="pvt", bufs=2)
                        for cp in range(NCH // 2):
                            nc.tensor.matmul(
                                pvt[:],
                                chs[:, 2 * cp : 2 * cp + 2, t * 128 : (t + 1) * 128],
                                wv_sb[:, 2 * cp : 2 * cp + 2, :, :],
                                start=(cp == 0), stop=(cp == NCH // 2 - 1),
                                perf_mode=DR,
                            )
                        kt = tb * (TB // 128) + t
                        for h in range(HPC):
                            with nc.allow_low_precision(reason="fp8 PV operand"):
                                nc.vector.tensor_copy(
                                    Vn[h][:, kt // 2, kt % 2, :],
                                    pvt[:, h * D : (h + 1) * D],
                                )

                def mk(cls, src, fns):
                    box = [None]
                    tb = fns[0][1]

                    def dma_thunk(box=box, src=src, tb=tb):
                        box[0] = chunk_dma(src, tb)

                    def mm_thunk(box=box, fns=fns):
                        for fn, tb, *rest in fns:
                            fn(tb, *rest, box)

                    units.append((b, cls, dma_thunk, mm_thunk))

                for tb in range(NTB):
                    mk("A", kT, [(k_unit, tb, 0)])
                for tb in range(NTB):
                    mk("A", qT, [(q_unit, tb, 0), (v_unit, tb)])
                for tb in range(NTB):
                    mk("B", kT, [(k_unit, tb, 1)])
                for tb in range(NTB):
                    mk("B", qT, [(q_unit, tb, 1)])
                return units

            # ---------- unit scheduler (dma issued 2 units ahead) ----------
            queue = deque()
            dma_lead = deque()
            LEAD = 2

            def _top_up():
                while queue and len(dma_lead) < LEAD:
                    u = queue.popleft()
                    u[2]()
                    dma_lead.append(u)

            def pull_one():
                _top_up()
                if dma_lead:
                    u = dma_lead.popleft()
                    u[3]()
                    _top_up()

            def flush(pred):
                _top_up()
                while dma_lead and pred(dma_lead[0]):
                    u = dma_lead.popleft()
                    u[3]()
                    _top_up()

            def push_units(b):
                for u in make_units(b):
                    queue.append(u)
                _top_up()

            # ---------- attention ----------
            def attn_batch(b):
                QT, KTs, Vn = QTd.pop(b), KTd.pop(b), Vnd.pop(b)
                for qbp in range(NQP):
                    q0 = qbp * 2 * QB
                    sl0 = slice(q0, q0 + QB)
                    sl1 = slice(q0 + QB, q0 + 2 * QB)
                    ytile = work.tile([128, 2 * QB], f32, tag="ytile", name="ytile")
                    for h in range(HPC):
                        if h == 1:
                            flush(lambda u: u[0] < b or (u[0] == b and u[1] == "B"))
                        pctx = psC.tile([128, 2 * QB], f32, tag="pCtx", name="pctx", bufs=1)
                        acc_d = work.tile([128, 2 * QB], bf16, tag="acc_d", name="acc_d")
                        st = [True, None]
                        for pair in range(NKP):
                            pexp2 = pexpp.tile([128, 2, 2 * QB], f8, tag="pexp", bufs=6)
                            for sub in range(2):
                                kt = 2 * pair + sub
                                ps2 = psS.tile([128, 2 * QB], f32, tag="pS", name="ps2", bufs=2)
                                ksl = slice(kt * 128, (kt + 1) * 128)
                                nc.tensor.matmul(
                                    ps2[:, :QB], KTs[h][:, ksl], QT[h][:, sl0],
                                    start=True, stop=True,
                                )
                                nc.tensor.matmul(
                                    ps2[:, QB:], KTs[h][:, ksl], QT[h][:, sl1],
                                    start=True, stop=True,
                                )
                                nc.scalar.activation(
                                    pexp2[:, sub, :], ps2[:], AF.Exp, scale=scale
                                )
                                if kt % 4 == 1:
                                    pull_one()  # proj filler where PE waits on exp
                                with nc.allow_low_precision(reason="bf16 rowsum acc"):
                                    if st[0] and st[1] is None:
                                        st[1] = pexp2[:, sub, :]
                                    elif st[0]:
                                        nc.vector.tensor_add(
                                            acc_d[:], st[1], pexp2[:, sub, :]
                                        )
                                        st[0] = False
                                    else:
                                        nc.vector.tensor_add(
                                            acc_d[:], acc_d[:], pexp2[:, sub, :]
                                        )
                            for half in range(2):
                                hs = slice(half * QB, (half + 1) * QB)
                                nc.tensor.matmul(
                                    pctx[:, hs], Vn[h][:, pair], pexp2[:, :, hs],
                                    start=(pair == 0), stop=(pair == NKP - 1),
                                    perf_mode=DR,
                                )
                        # rowsum collapse + normalize
                        rsbr = work.tile([128, 2 * QB], f32, tag="rsbr", name="rsbr", bufs=2)
                        for half in range(2):
                            hs = slice(half * QB, (half + 1) * QB)
                            pbc = psP.tile([128, QB], f32, tag="pP", name="pbc", bufs=2)
                            nc.tensor.matmul(
                                pbc[:], onemb_sb[:], acc_d[:, hs], start=True, stop=True
                            )
                            nc.vector.reciprocal_approx_fast(out=rsbr[:, hs], in_=pbc[:])
                        pull_one()
                        if h == 0:
                            for half in range(2):
                                hs = slice(half * QB, (half + 1) * QB)
                                nc.vector.tensor_mul(ytile[:, hs], pctx[:, hs], rsbr[:, hs])
                        else:
                            ctxn = work.tile([128, 2 * QB], f32, tag="ctxn", name="ctxn")
                            for half in range(2):
                                hs = slice(half * QB, (half + 1) * QB)
                                nc.vector.tensor_mul(ctxn[:, hs], pctx[:, hs], rsbr[:, hs])
                                nc.vector.tensor_add(
                                    ytile[:, hs], ytile[:, hs], ctxn[:, hs]
                                )
                                nc.gpsimd.dma_start(
                                    y_bounce[b][qbp][:, hs], ytile[:, hs]
                                )
                            nc.gpsimd.collective_compute(
                                "ReduceScatter",
                                mybir.AluOpType.add,
                                replica_groups=[list(range(NCORES))],
                                ins=[y_bounce[b][qbp][:].opt()],
                                outs=[y_shard[b][qbp][:].opt()],
                            )
                            nc.gpsimd.dma_start(out_y[b, qbp], y_shard[b][qbp][:])

            # ---------- schedule ----------
            push_units(0)
            flush(lambda u: u[0] == 0 and u[1] == "A")
            for b in range(B):
                if b + 1 < B:
                    push_units(b + 1)
                attn_batch(b)
                flush(lambda u: u[0] <= b)
            flush(lambda u: True)

    nc.compile()
    return nc


def kernel(**inputs):
    query = np.asarray(inputs["query"], np.float32)
    key = np.asarray(inputs["key"], np.float32)
    Wq, bq = np.asarray(inputs["Wq"], np.float32), np.asarray(inputs["bq"], np.float32)
    Wk, bk = np.asarray(inputs["Wk"], np.float32), np.asarray(inputs["bk"], np.float32)
    Wv, bv = np.asarray(inputs["Wv"], np.float32), np.asarray(inputs["bv"], np.float32)
    Wp, bp = np.asarray(inputs["Wp"], np.float32), np.asarray(inputs["bp"], np.float32)
    Wo, bo = np.asarray(inputs["Wo"], np.float32), np.asarray(inputs["bo"], np.float32)

    f8np = ml_dtypes.float8_e4m3
    qT_8 = np.ascontiguousarray(query.transpose(0, 2, 1)).astype(f8np)
    kT_8 = np.ascontiguousarray(key.transpose(0, 2, 1)).astype(f8np)

    if "nc" not in _cache:
        _cache["nc"] = build()
    nc = _cache["nc"]

    def prepack_qk(w):  # 32x-scaled [HPC, IN, D] -> [128, HPC, NCH, D] fp8
        return np.ascontiguousarray(
            (32.0 * w).reshape(HPC, NCH, 128, D).transpose(2, 0, 1, 3)
        ).astype(f8np)

    def prepack_v(w):  # 64x-scaled [HPC, IN, D] -> [128, NCH, HPC, D] fp8
        return np.ascontiguousarray(
            (64.0 * w).reshape(HPC, NCH, 128, D).transpose(2, 1, 0, 3)
        ).astype(f8np)

    Wo_h = Wo.reshape(H, D, D)
    bias_total = (
        np.einsum("hd,hde,hef->f", bv.astype(np.float64), Wp.astype(np.float64), Wo_h.astype(np.float64))
        + np.einsum("hd,hdf->f", bp.astype(np.float64), Wo_h.astype(np.float64))
        + bo.astype(np.float64)
    ).astype(np.float32)

    in_maps = []
    for i in range(NCORES):
        hs = slice(i * HPC, (i + 1) * HPC)
        wvpp = np.einsum(
            "hid,hde,hef->hif",
            Wv[hs].astype(np.float64),
            Wp[hs].astype(np.float64),
            Wo_h[hs].astype(np.float64),
        ).astype(np.float32)
        in_maps.append(
            {
                "qT": qT_8,
                "kT": kT_8,
                "wq": prepack_qk(Wq[hs]),
                "wk": prepack_qk(Wk[hs]),
                "wv": prepack_v(wvpp),
                "bqT": np.ascontiguousarray(32.0 * bq[hs].T),
                "bkT": np.ascontiguousarray(32.0 * bk[hs].T),
                "onemb": np.ones((D, D), ml_dtypes.bfloat16),
            }
        )

    res = run_bass_kernel_spmd(nc, in_maps, core_ids=list(range(NCORES)))
    _cache["last_result"] = res
    # shards: per core [B, NQP, ESH, 2QB] -> full [B, S, D]; y is 64x-scaled
    parts = np.stack([res.results[i]["out_y"] for i in range(NCORES)], axis=2)
    yfull = parts.reshape(B, NQP, D, 2 * QB).transpose(0, 1, 3, 2).reshape(B, S, D)
    return np.ascontiguousarray(yfull / 64.0 + bias_total[None, None, :])


# revision 10
# speedup vs baseline: 1.5999x; 1.0117x over previous
"""Multi-head attention on 8 trn2 NeuronCores, head-parallel (2 heads/core).

Math per head h (reference semantics):
  Q = query @ Wq[h] + bq[h];  K = key @ Wk[h] + bk[h];  V = query @ Wv[h] + bv[h]
  P = exp(Q K^T / sqrt(D));  alpha = P / rowsum(P)
  ctx = alpha @ V;  y_h = (ctx @ Wp[h] + bp[h]) @ Wo[h]
  out = sum_h y_h + bo

Device-side formulation:
  Rows of alpha sum to 1, so all linear tails fold into the V projection:
    out = sum_h alpha_h @ (X Wv_h Wp_h Wo_h) + const_bias_row
  Projections and the PV contraction run in fp8-e4m3 with DoubleRow perf
  mode (2 k-tiles per pass = 2x PE throughput); QK^T stays bf16 (its
  contraction is a single 128 k-tile).  Weights are host-scaled into the
  fp8 normal range (wq,wk x32 -> exp scale /1024; wv''=Wv Wp Wo x64 ->
  host output /64).  Unnormalized softmax; rowsum via ones-matmul
  collapse of a DVE-accumulated exp sum; y = sum_h ctx_h / rowsum_h;
  ReduceScatter per 1024-query block; host adds the bias row.

Scheduling: the attention loop is ACT(exp)-bound while projections are
pure-PE, so projection work for batch b+1 is interleaved into the
attention kt-loops of batch b as "units".  DMA issue cost (~600ns per
dma_start on the issuing sequencer) means few, large dma ops.  Queues:
input chunks on sync, y staging on vector, collectives + out dma on
gpsimd (so a waiting ReduceScatter never blocks input DMAs).
"""

import sys

if "/opt/trn_rl_repo" not in sys.path:
    sys.path.insert(0, "/opt/trn_rl_repo")

from collections import deque

import ml_dtypes
import numpy as np

import concourse.mybir as mybir
import concourse.tile as tile
from concourse import bacc
from concourse.bass_utils import run_bass_kernel_spmd

B, S = 4, 2048
IN, D, H = 1024, 128, 16
NCORES = 8
HPC = H // NCORES  # heads per core
NCH = IN // 128  # input chunks
TB = 512  # projection token block
NTB = S // TB
QB = 512  # attention query block
NQB = S // QB
NKT = S // 128  # attention key tiles
NKP = NKT // 2  # key-tile pairs (DoubleRow)
ESH = D // NCORES  # output shard rows per core
NQP = NQB // 2

f32 = mybir.dt.float32
bf16 = mybir.dt.bfloat16
f8 = mybir.dt.float8e4
AF = mybir.ActivationFunctionType
DR = mybir.MatmulPerfMode.DoubleRow

_cache = {}


def build():
    nc = bacc.Bacc(None, target_bir_lowering=False, num_devices=NCORES)

    qT = nc.dram_tensor("qT", [B, IN, S], f8, kind="ExternalInput")
    kT = nc.dram_tensor("kT", [B, IN, S], f8, kind="ExternalInput")
    # prepacked, host-scaled fp8 weights (see kernel())
    wq = nc.dram_tensor("wq", [128, HPC, NCH, D], f8, kind="ExternalInput")
    wk = nc.dram_tensor("wk", [128, HPC, NCH, D], f8, kind="ExternalInput")
    wv = nc.dram_tensor("wv", [128, NCH, HPC, D], f8, kind="ExternalInput")
    bqT = nc.dram_tensor("bqT", [D, HPC], f32, kind="ExternalInput")
    bkT = nc.dram_tensor("bkT", [D, HPC], f32, kind="ExternalInput")
    onemb = nc.dram_tensor("onemb", [D, D], bf16, kind="ExternalInput")

    out_y = nc.dram_tensor("out_y", [B, NQP, ESH, 2 * QB], f32, kind="ExternalOutput")
    y_bounce = [
        [nc.dram_tensor(f"y_bounce{b}_{q}", [D, 2 * QB], f32) for q in range(NQP)]
        for b in range(B)
    ]
    y_shard = [
        [nc.dram_tensor(f"y_shard{b}_{q}", [ESH, 2 * QB], f32) for q in range(NQP)]
        for b in range(B)
    ]

    # Q,K are x32-scaled -> scores x1024
    scale = 1.0 / float(np.sqrt(D)) / 1024.0

    with tile.TileContext(nc) as tc:
        with (
            tc.tile_pool(name="const", bufs=1) as cpool,
            tc.tile_pool(name="xch", bufs=3) as xch,
            tc.tile_pool(name="qkv", bufs=2) as qkv,
            tc.tile_pool(name="work", bufs=2) as work,
            tc.tile_pool(name="pexpp", bufs=6) as pexpp,
            tc.tile_pool(name="psS", bufs=2, space="PSUM") as psS,
            tc.tile_pool(name="psC", bufs=1, space="PSUM") as psC,
            tc.tile_pool(name="psP", bufs=2, space="PSUM") as psP,
        ):
            # ---- resident constants (one DMA op each) ----
            wq_sb = cpool.tile([128, HPC, NCH, D], f8, tag="wq_sb")
            wk_sb = cpool.tile([128, HPC, NCH, D], f8, tag="wk_sb")
            wv_sb = cpool.tile([128, NCH, HPC, D], f8, tag="wv_sb")
            for sb_t, dram_t in ((wq_sb, wq), (wk_sb, wk), (wv_sb, wv)):
                nc.sync.dma_start(sb_t[:], dram_t[:])
            bq_sb = cpool.tile([128, HPC], f32, tag="bq_sb")
            bk_sb = cpool.tile([128, HPC], f32, tag="bk_sb")
            nc.sync.dma_start(bq_sb[:], bqT[:])
            nc.sync.dma_start(bk_sb[:], bkT[:])
            onemb_sb = cpool.tile([D, D], bf16, tag="onemb_sb")
            nc.sync.dma_start(onemb_sb[:], onemb[:])

            QTd, KTd, Vnd = {}, {}, {}

            # ---------- projection units ----------
            # Each unit: (batch, cls, dma_thunk, mm_thunk).  cls 'A' must land
            # before attn(batch) starts; cls 'B' (head-1 Q/K) before the first
            # h==1 attention section of the batch.
            def make_units(b):
                QT = QTd[b] = [qkv.tile([128, S], bf16, tag=f"QT{h}", name=f"QT{b}_{h}") for h in range(HPC)]
                KTs = KTd[b] = [qkv.tile([128, S], bf16, tag=f"KT{h}", name=f"KT{b}_{h}") for h in range(HPC)]
                Vn = Vnd[b] = [
                    qkv.tile([128, NKP, 2, 128], f8, tag=f"VN{h}", name=f"VN{b}_{h}")
                    for h in range(HPC)
                ]
                units = []

                def chunk_dma(src, tb):
                    chs = xch.tile([128, NCH, TB], f8, tag="xch", bufs=3)
                    sl = slice(tb * TB, (tb + 1) * TB)
                    nc.sync.dma_start(
                        chs[:], src[b, :, sl].rearrange("(c p) n -> p c n", p=128)
                    )
                    return chs

                def qk_unit(tb, h, w_sb, bias_sb, dst, box):
                    sl = slice(tb * TB, (tb + 1) * TB)
                    chs = box[0]
                    pq = psP.tile([128, TB], f32, tag="pP", name="pqk", bufs=2)
                    for cp in range(NCH // 2):
                        nc.tensor.matmul(
                            pq[:],
                            w_sb[:, h, 2 * cp : 2 * cp + 2, :],
                            chs[:, 2 * cp : 2 * cp + 2, :],
                            start=(cp == 0), stop=(cp == NCH // 2 - 1),
                            perf_mode=DR,
                        )
                    with nc.allow_low_precision(reason="f32 psum -> bf16"):
                        nc.vector.tensor_scalar_add(
                            dst[h][:, sl], pq[:], bias_sb[:, h : h + 1]
                        )

                def k_unit(tb, h, box):
                    qk_unit(tb, h, wk_sb, bk_sb, KTs, box)

                def q_unit(tb, h, box):
                    qk_unit(tb, h, wq_sb, bq_sb, QT, box)

                def v_unit(tb, box):
                    chs = box[0]
                    for t in range(TB // 128):
                        pvt = psP.tile([128, 2 * D], f32, tag="pP", name="pvt", bufs=2)
                        for cp in range(NCH // 2):
                            nc.tensor.matmul(
                                pvt[:],
                                chs[:, 2 * cp : 2 * cp + 2, t * 128 : (t + 1) * 128],
                                wv_sb[:, 2 * cp : 2 * cp + 2, :, :],
                                start=(cp == 0), stop=(cp == NCH // 2 - 1),
                                perf_mode=DR,
                            )
                        kt = tb * (TB // 128) + t
                        for h in range(HPC):
                            with nc.allow_low_precision(reason="fp8 PV operand"):
                                nc.vector.tensor_copy(
                                    Vn[h][:, kt // 2, kt % 2, :],
                                    pvt[:, h * D : (h + 1) * D],
                                )

                def mk(cls, src, fns):
                    box = [None]
                    tb = fns[0][1]

                    def dma_thunk(box=box, src=src, tb=tb):
                        box[0] = chunk_dma(src, tb)

                    def mm_thunk(box=box, fns=fns):
                        for fn, tb, *rest in fns:
                            fn(tb, *rest, box)

                    units.append((b, cls, dma_thunk, mm_thunk))

                for tb in range(NTB):
                    mk("A", kT, [(k_unit, tb, 0)])
                for tb in range(NTB):
                    mk("A", qT, [(q_unit, tb, 0), (v_unit, tb)])
                for tb in range(NTB):
                    mk("B", kT, [(k_unit, tb, 1)])
                for tb in range(NTB):
                    mk("B", qT, [(q_unit, tb, 1)])
                return units

            # ---------- unit scheduler (dma issued 2 units ahead) ----------
            queue = deque()
            dma_lead = deque()
            LEAD = 2

            def _top_up():
                while queue and len(dma_lead) < LEAD:
                    u = queue.popleft()
                    u[2]()
                    dma_lead.append(u)

            def pull_one():
                _top_up()
                if dma_lead:
                    u = dma_lead.popleft()
                    u[3]()
                    _top_up()

            def flush(pred):
                _top_up()
                while dma_lead and pred(dma_lead[0]):
                    u = dma_lead.popleft()
                    u[3]()
                    _top_up()

            def push_units(b):
                for u in make_units(b):
                    queue.append(u)
                _top_up()

            # ---------- attention ----------
            def attn_batch(b):
                QT, KTs, Vn = QTd.pop(b), KTd.pop(b), Vnd.pop(b)
                for qbp in range(NQP):
                    q0 = qbp * 2 * QB
                    sl0 = slice(q0, q0 + QB)
                    sl1 = slice(q0 + QB, q0 + 2 * QB)
                    ytile = work.tile([128, 2 * QB], f32, tag="ytile", name="ytile")
                    for h in range(HPC):
                        if h == 1:
                            flush(lambda u: u[0] < b or (u[0] == b and u[1] == "B"))
                        pctx = psC.tile([128, 2 * QB], f32, tag="pCtx", name="pctx", bufs=1)
                        # rowsum accumulators: pair-wide adds split across the
                        # vector and gpsimd engines (DVE alone can't keep up
                        # with fp8-rate adds)
                        acc_v = work.tile([128, 2, 2 * QB], bf16, tag="acc_v", name="acc_v")
                        acc_g = work.tile([128, 2, 2 * QB], bf16, tag="acc_g", name="acc_g")
                        ptiles = []
                        for pair in range(NKP):
                            pexp2 = pexpp.tile([128, 2, 2 * QB], f8, tag="pexp", bufs=8)
                            ptiles.append(pexp2)
                            for sub in range(2):
                                kt = 2 * pair + sub
                                ps2 = psS.tile([128, 2 * QB], f32, tag="pS", name="ps2", bufs=2)
                                ksl = slice(kt * 128, (kt + 1) * 128)
                                nc.tensor.matmul(
                                    ps2[:, :QB], KTs[h][:, ksl], QT[h][:, sl0],
                                    start=True, stop=True,
                                )
                                nc.tensor.matmul(
                                    ps2[:, QB:], KTs[h][:, ksl], QT[h][:, sl1],
                                    start=True, stop=True,
                                )
                                nc.scalar.activation(
                                    pexp2[:, sub, :], ps2[:], AF.Exp, scale=scale
                                )
                                if kt % 4 == 1:
                                    pull_one()  # proj filler where PE waits on exp
                            for half in range(2):
                                hs = slice(half * QB, (half + 1) * QB)
                                nc.tensor.matmul(
                                    pctx[:, hs], Vn[h][:, pair], pexp2[:, :, hs],
                                    start=(pair == 0), stop=(pair == NKP - 1),
                                    perf_mode=DR,
                                )
                            with nc.allow_low_precision(reason="bf16 rowsum acc"):
                                if pair == 2:
                                    nc.vector.tensor_add(acc_v[:], ptiles[0][:], ptiles[2][:])
                                elif pair == 3:
                                    nc.gpsimd.tensor_add(acc_g[:], ptiles[1][:], ptiles[3][:])
                                elif pair in (4, 6):
                                    nc.vector.tensor_add(acc_v[:], acc_v[:], ptiles[pair][:])
                                elif pair in (5, 7):
                                    nc.gpsimd.tensor_add(acc_g[:], acc_g[:], ptiles[pair][:])
                        # rowsum collapse + normalize
                        rsbr = work.tile([128, 2 * QB], f32, tag="rsbr", name="rsbr", bufs=2)
                        for half in range(2):
                            hs = slice(half * QB, (half + 1) * QB)
                            pbc = psP.tile([128, QB], f32, tag="pP", name="pbc", bufs=2)
                            srcs = [acc_v[:, 0, hs], acc_v[:, 1, hs], acc_g[:, 0, hs], acc_g[:, 1, hs]]
                            for si, src in enumerate(srcs):
                                nc.tensor.matmul(
                                    pbc[:], onemb_sb[:], src,
                                    start=(si == 0), stop=(si == len(srcs) - 1),
                                )
                            nc.vector.reciprocal_approx_fast(out=rsbr[:, hs], in_=pbc[:])
                        pull_one()
                        if h == 0:
                            for half in range(2):
                                hs = slice(half * QB, (half + 1) * QB)
                                nc.vector.tensor_mul(ytile[:, hs], pctx[:, hs], rsbr[:, hs])
                        else:
                            ctxn = work.tile([128, 2 * QB], f32, tag="ctxn", name="ctxn")
                            for half in range(2):
                                hs = slice(half * QB, (half + 1) * QB)
                                nc.vector.tensor_mul(ctxn[:, hs], pctx[:, hs], rsbr[:, hs])
                                nc.vector.tensor_add(
                                    ytile[:, hs], ytile[:, hs], ctxn[:, hs]
                                )
                                nc.sync.dma_start(
                                    y_bounce[b][qbp][:, hs], ytile[:, hs]
                                )
                            nc.gpsimd.collective_compute(
                                "ReduceScatter",
                                mybir.AluOpType.add,
                                replica_groups=[list(range(NCORES))],
                                ins=[y_bounce[b][qbp][:].opt()],
                                outs=[y_shard[b][qbp][:].opt()],
                            )

            # ---------- schedule ----------
            push_units(0)
            flush(lambda u: u[0] == 0 and u[1] == "A")
            for b in range(B):
                if b + 1 < B:
                    push_units(b + 1)
                attn_batch(b)
                flush(lambda u: u[0] <= b)
            flush(lambda u: True)
            # out dmas at the end: each waits its RS; nothing queues behind
            for b in range(B):
                for qbp in range(NQP):
                    nc.sync.dma_start(out_y[b, qbp], y_shard[b][qbp][:])

    nc.compile()
    return nc


def kernel(**inputs):
    query = np.asarray(inputs["query"], np.float32)
    key = np.asarray(inputs["key"], np.float32)
    Wq, bq = np.asarray(inputs["Wq"], np.float32), np.asarray(inputs["bq"], np.float32)
    Wk, bk = np.asarray(inputs["Wk"], np.float32), np.asarray(inputs["bk"], np.float32)
    Wv, bv = np.asarray(inputs["Wv"], np.float32), np.asarray(inputs["bv"], np.float32)
    Wp, bp = np.asarray(inputs["Wp"], np.float32), np.asarray(inputs["bp"], np.float32)
    Wo, bo = np.asarray(inputs["Wo"], np.float32), np.asarray(inputs["bo"], np.float32)

    f8np = ml_dtypes.float8_e4m3
    qT_8 = np.ascontiguousarray(query.transpose(0, 2, 1)).astype(f8np)
    kT_8 = np.ascontiguousarray(key.transpose(0, 2, 1)).astype(f8np)

    if "nc" not in _cache:
        _cache["nc"] = build()
    nc = _cache["nc"]

    def prepack_qk(w):  # 32x-scaled [HPC, IN, D] -> [128, HPC, NCH, D] fp8
        return np.ascontiguousarray(
            (32.0 * w).reshape(HPC, NCH, 128, D).transpose(2, 0, 1, 3)
        ).astype(f8np)

    def prepack_v(w):  # 64x-scaled [HPC, IN, D] -> [128, NCH, HPC, D] fp8
        return np.ascontiguousarray(
            (64.0 * w).reshape(HPC, NCH, 128, D).transpose(2, 1, 0, 3)
        ).astype(f8np)

    Wo_h = Wo.reshape(H, D, D)
    bias_total = (
        np.einsum("hd,hde,hef->f", bv.astype(np.float64), Wp.astype(np.float64), Wo_h.astype(np.float64))
        + np.einsum("hd,hdf->f", bp.astype(np.float64), Wo_h.astype(np.float64))
        + bo.astype(np.float64)
    ).astype(np.float32)

    in_maps = []
    for i in range(NCORES):
        hs = slice(i * HPC, (i + 1) * HPC)
        wvpp = np.einsum(
            "hid,hde,hef->hif",
            Wv[hs].astype(np.float64),
            Wp[hs].astype(np.float64),
            Wo_h[hs].astype(np.float64),
        ).astype(np.float32)
        in_maps.append(
            {
                "qT": qT_8,
                "kT": kT_8,
                "wq": prepack_qk(Wq[hs]),
                "wk": prepack_qk(Wk[hs]),
                "wv": prepack_v(wvpp),
                "bqT": np.ascontiguousarray(32.0 * bq[hs].T),
                "bkT": np.ascontiguousarray(32.0 * bk[hs].T),
                "onemb": np.ones((D, D), ml_dtypes.bfloat16),
            }
        )

    res = run_bass_kernel_spmd(nc, in_maps, core_ids=list(range(NCORES)))
    _cache["last_result"] = res
    # shards: per core [B, NQP, ESH, 2QB] -> full [B, S, D]; y is 64x-scaled
    parts = np.stack([res.results[i]["out_y"] for i in range(NCORES)], axis=2)
    yfull = parts.reshape(B, NQP, D, 2 * QB).transpose(0, 1, 3, 2).reshape(B, S, D)
    return np.ascontiguousarray(yfull / 64.0 + bias_total[None, None, :])
